# revision 1
# baseline (speedup 1.0000x reference)
"""AttentionBlock Trainium2 kernel (Bass/Tile), SPMD over 8 NeuronCores.

Problem (hardcoded): x [32, 256, 32, 32] fp32
  GroupNorm(8 groups, eps=1e-5, affine) -> 1x1 qkv conv [768,256] ->
  per-image attention over N=1024 pixels (C=256) -> 1x1 proj [256,256] ->
  residual add.

Sharding: pure data-parallel over batch: 4 images per core, weights
replicated, no collectives.

Per-image on-chip schedule (channels-on-partitions layout, bf16 matmuls
with fp32 PSUM accumulation and fp32 statistics):
  - GroupNorm stats via bn_stats/bn_aggr (per-channel, fp32), pooled over
    each group's 32 channels with a tiny mask-matmul, finalized at group
    level in fp32, then broadcast back to channels via a DRAM-bounce DMA.
  - norm_w/norm_b are folded into the qkv weights on the host, the qk
    1/sqrt(C) scale is folded into Wq/Wk, v/proj biases fold into one
    final per-channel bias.
  - Attention is computed transposed: S^T[k,q] = k^T q directly from the
    conv layout (no transposes anywhere), softmax without max-subtraction
    (|S| <= ~6 by construction), Z via a ones-matmul in PSUM, O = v_T^T P
    in PSUM accumulation over k-blocks. The reciprocal of Z is computed on
    a [128, 4]-transposed copy (via a DRAM bounce; a [1, 512] one-lane
    reciprocal costs ~4 us on DVE) and broadcast back over partitions.
  - proj commutes with the per-q-column 1/Z scale, so proj runs directly
    on (a bf16 copy of) O and the 1/Z bounce only gates the final DVE
    multiply-add: y = x + proj(O) * (1/Z) + bias.

The per-image work is software-pipelined at emission order (per-engine
instruction streams execute in order, so emission order is the schedule):
image b+1's x-load/stats run on DVE under image b's attention matmuls,
S(nb+1) is emitted before the exp-gated O(nb) so the PE never waits on
ScalarE, and each q-chunk's 1/Z DRAM bounce is hidden under the next
chunk's matmuls (phase_d deferred by one chunk). A few dep-free warm-up
matmuls after the weight DMAs un-throttle the PE clock (HAM) before the
first real compute.

Measured on 8 axon trn2 cores: ~185-212 us HW exec, rel err ~4.1e-4.
"""

from contextlib import ExitStack

import ml_dtypes
import numpy as np

import concourse.bass as bass
import concourse.tile as tile
from concourse import bacc
from concourse import mybir

F32 = mybir.dt.float32
BF16 = mybir.dt.bfloat16
AF = mybir.ActivationFunctionType
OP = mybir.AluOpType

B, C, H, W = 32, 256, 32, 32
N = H * W            # 1024
G = 8                # groups
EPS = 1e-5
NCORES = 8
BL = B // NCORES     # images per core
CT = C // 128        # channel tiles
NB = N // 128        # pixel blocks (k dim of attention)
QCH = N // 512       # 512-wide q chunks
P = 128
RZ_SPLIT = True
import os as _os
Z_MODE = _os.environ.get("KERNEL_Z_MODE", "pe")  # 'pe' | 'hybrid'
N_WARM = int(_os.environ.get("KERNEL_N_WARM", "24"))


def build_program(use_bq: bool, use_bk: bool, use_bf: bool) -> bass.Bass:
    nc = bacc.Bacc()

    xs = nc.dram_tensor("xs", [BL, C, N], F32, kind="ExternalInput")
    wq = nc.dram_tensor("wq", [C, C], BF16, kind="ExternalInput")  # [c_in, c_out]
    wk = nc.dram_tensor("wk", [C, C], BF16, kind="ExternalInput")
    wv = nc.dram_tensor("wv", [C, C], BF16, kind="ExternalInput")
    wp = nc.dram_tensor("wp", [C, C], BF16, kind="ExternalInput")
    bq = nc.dram_tensor("bq", [C], F32, kind="ExternalInput")
    bk = nc.dram_tensor("bk", [C], F32, kind="ExternalInput")
    bf = nc.dram_tensor("bf", [C], F32, kind="ExternalInput")
    out = nc.dram_tensor("out", [BL, C, N], F32, kind="ExternalOutput")

    # Constant matrix for the group-stat pooling matmul (mean over each
    # group's 32 channels; 1/32 is exact in bf16).
    gmask_np = np.zeros((P, 4), np.float32)
    gmask_np[np.arange(P), np.arange(P) // 32] = 1.0 / 32.0
    gmask_d = nc.inline_tensor(gmask_np.astype(ml_dtypes.bfloat16), "gmask")

    with tile.TileContext(nc) as tc, ExitStack() as ctx:
        consts = ctx.enter_context(tc.tile_pool(name="consts", bufs=1))
        xpool = ctx.enter_context(tc.tile_pool(name="xp", bufs=3))
        hpool = ctx.enter_context(tc.tile_pool(name="hp", bufs=2))
        qpool = ctx.enter_context(tc.tile_pool(name="qp", bufs=2))
        kpool = ctx.enter_context(tc.tile_pool(name="kp", bufs=2))
        vpool = ctx.enter_context(tc.tile_pool(name="vp", bufs=2))
        ppool = ctx.enter_context(tc.tile_pool(name="pp", bufs=3))
        opool = ctx.enter_context(tc.tile_pool(name="op", bufs=2))
        spool = ctx.enter_context(tc.tile_pool(name="sp", bufs=2))
        rzpool = ctx.enter_context(tc.tile_pool(name="rzp", bufs=2))
        outp = ctx.enter_context(tc.tile_pool(name="outp", bufs=4))
        dram = ctx.enter_context(tc.tile_pool(name="dram", bufs=2, space="DRAM"))
        psw = ctx.enter_context(tc.tile_pool(name="psw", bufs=4, space="PSUM"))
        psO = ctx.enter_context(tc.tile_pool(name="psO", bufs=1, space="PSUM"))
        psz = ctx.enter_context(tc.tile_pool(name="psz", bufs=2, space="PSUM"))

        # --- constants ---
        gmask_sb = consts.tile([P, 4], BF16, tag="gmask")
        nc.sync.dma_start(out=gmask_sb, in_=gmask_d[:, :])
        bq_sb = consts.tile([P, CT], F32, tag="bq")
        nc.sync.dma_start(out=bq_sb, in_=bq[:].rearrange("(t p) -> p t", p=P))
        bk_sb = consts.tile([P, CT], F32, tag="bk")
        nc.sync.dma_start(out=bk_sb, in_=bk[:].rearrange("(t p) -> p t", p=P))
        bf_sb = consts.tile([P, CT], F32, tag="bf")
        nc.sync.dma_start(out=bf_sb, in_=bf[:].rearrange("(t p) -> p t", p=P))
        onesc_sb = consts.tile([P, 1], F32, tag="onesc")
        nc.vector.memset(onesc_sb, 1.0)
        onesc_bf_sb = consts.tile([P, 1], BF16, tag="onescbf")
        nc.vector.memset(onesc_bf_sb, 1.0)
        eps_sb = consts.tile([P, 1], F32, tag="eps")
        nc.vector.memset(eps_sb, EPS)
        wq_sb = consts.tile([P, CT, C], BF16, tag="wq")
        wk_sb = consts.tile([P, CT, C], BF16, tag="wk")
        wv_sb = consts.tile([P, CT, C], BF16, tag="wv")
        wp_sb = consts.tile([P, CT, C], BF16, tag="wp")

        def load_weights():
            for t_sb, t_d in ((wq_sb, wq), (wk_sb, wk), (wv_sb, wv), (wp_sb, wp)):
                nc.sync.dma_start(
                    out=t_sb, in_=t_d[:, :].rearrange("(t p) o -> p t o", p=P)
                )

        # Per-image state carried between pipeline phases.
        st = [dict() for _ in range(BL)]

        def phase_a(b):
            """Load x, GroupNorm stats -> per-channel (mean, rstd), h."""
            x_t = xpool.tile([P, CT, N], F32, tag="x")
            st[b]["x"] = x_t
            for ct in range(CT):
                nc.sync.dma_start(
                    out=x_t[:, ct, :], in_=xs[b, ct * P : (ct + 1) * P, :]
                )
            chst = spool.tile([P, 2 * CT], F32, tag="chst")
            for ct in range(CT):
                bnst = spool.tile([P, 2, 6], F32, tag="bnst")
                for s in range(2):
                    nc.vector.bn_stats(
                        out=bnst[:, s, :], in_=x_t[:, ct, s * 512 : (s + 1) * 512]
                    )
                nc.vector.bn_aggr(out=chst[:, 2 * ct : 2 * ct + 2], in_=bnst)
                msq = spool.tile([P, 1], F32, tag="msq")
                nc.vector.tensor_mul(
                    out=msq,
                    in0=chst[:, 2 * ct : 2 * ct + 1],
                    in1=chst[:, 2 * ct : 2 * ct + 1],
                )
                nc.vector.tensor_add(
                    out=chst[:, 2 * ct + 1 : 2 * ct + 2],
                    in0=chst[:, 2 * ct + 1 : 2 * ct + 2],
                    in1=msq,
                )
            chst_bf = spool.tile([P, 2 * CT], BF16, tag="chstbf")
            nc.vector.tensor_copy(out=chst_bf, in_=chst)
            gst_ps = psw.tile([4, 2 * CT], F32, tag="w")
            nc.tensor.matmul(
                gst_ps, lhsT=gmask_sb, rhs=chst_bf, start=True, stop=True
            )
            gst_sb = spool.tile([4, 2 * CT], F32, tag="gst")
            nc.vector.tensor_copy(out=gst_sb, in_=gst_ps)
            gvar = spool.tile([4, CT], F32, tag="gvar")
            for ct in range(CT):
                gmsq = spool.tile([4, 1], F32, tag="gmsq")
                nc.vector.tensor_mul(
                    out=gmsq,
                    in0=gst_sb[:, 2 * ct : 2 * ct + 1],
                    in1=gst_sb[:, 2 * ct : 2 * ct + 1],
                )
                nc.vector.tensor_tensor(
                    out=gvar[:, ct : ct + 1],
                    in0=gst_sb[:, 2 * ct + 1 : 2 * ct + 2],
                    in1=gmsq,
                    op=OP.subtract,
                )
            gsd = spool.tile([4, CT], F32, tag="gsd")
            nc.scalar.activation(
                out=gsd, in_=gvar, func=AF.Sqrt, bias=eps_sb[0:4], scale=1.0
            )
            grstd = spool.tile([4, CT], F32, tag="grstd")
            nc.vector.reciprocal(out=grstd, in_=gsd)
            gfin = spool.tile([4, 2 * CT], F32, tag="gfin")
            for ct in range(CT):
                nc.vector.tensor_copy(
                    out=gfin[:, 2 * ct : 2 * ct + 1],
                    in_=gst_sb[:, 2 * ct : 2 * ct + 1],
                )
                nc.vector.tensor_copy(
                    out=gfin[:, 2 * ct + 1 : 2 * ct + 2],
                    in_=grstd[:, ct : ct + 1],
                )
            gfin_d = dram.tile([4, 2 * CT], F32, tag="gfd")
            nc.sync.dma_start(out=gfin_d, in_=gfin)
            pcs = spool.tile([P, 2 * CT], F32, tag="pcs")
            for g in range(4):
                nc.sync.dma_start(
                    out=pcs[32 * g : 32 * (g + 1), :],
                    in_=gfin_d[g : g + 1, :].to_broadcast((32, 2 * CT)),
                )
            h_t = hpool.tile([P, CT, N], BF16, tag="h")
            st[b]["h"] = h_t
            for ct in range(CT):
                nc.vector.tensor_scalar(
                    out=h_t[:, ct, :],
                    in0=x_t[:, ct, :],
                    scalar1=pcs[:, 2 * ct : 2 * ct + 1],
                    scalar2=pcs[:, 2 * ct + 1 : 2 * ct + 2],
                    op0=OP.subtract,
                    op1=OP.mult,
                )

        def phase_b(b):
            """qkv 1x1 convs."""
            h_t = st[b]["h"]
            q_sb = qpool.tile([P, CT, N], BF16, tag="q")
            k_sb = kpool.tile([P, CT, N], BF16, tag="k")
            st[b]["q"], st[b]["k"] = q_sb, k_sb
            for dst, w_sb, b_sb, use_b, on_act in (
                (q_sb, wq_sb, bq_sb, use_bq, True),
                (k_sb, wk_sb, bk_sb, use_bk, False),
            ):
                for ct in range(CT):
                    for nch in range(2):
                        mm_ps = psw.tile([P, 512], F32, tag="w")
                        for kc in range(CT):
                            nc.tensor.matmul(
                                mm_ps,
                                lhsT=w_sb[:, kc, ct * P : (ct + 1) * P],
                                rhs=h_t[:, kc, nch * 512 : (nch + 1) * 512],
                                start=(kc == 0),
                                stop=(kc == CT - 1),
                            )
                        dst_ap = dst[:, ct, nch * 512 : (nch + 1) * 512]
                        if use_b:
                            nc.vector.tensor_scalar_add(
                                out=dst_ap, in0=mm_ps, scalar1=b_sb[:, ct : ct + 1]
                            )
                        elif on_act:
                            nc.scalar.activation(
                                out=dst_ap, in_=mm_ps, func=AF.Copy, bias=0.0,
                                scale=1.0,
                            )
                        else:
                            nc.vector.tensor_copy(out=dst_ap, in_=mm_ps)
            v_sb = vpool.tile([P, NB, C], BF16, tag="v")
            st[b]["v"] = v_sb
            for nb in range(NB):
                vv_ps = psw.tile([P, C], F32, tag="w")
                for kc in range(CT):
                    nc.tensor.matmul(
                        vv_ps,
                        lhsT=h_t[:, kc, nb * P : (nb + 1) * P],
                        rhs=wv_sb[:, kc, :],
                        start=(kc == 0),
                        stop=(kc == CT - 1),
                    )
                nc.vector.tensor_copy(out=v_sb[:, nb, :], in_=vv_ps)

        def phase_c(b, qc):
            """Attention core for one 512-wide q chunk: S, exp, Z, O, 1/Z."""
            q_sb, k_sb, v_sb = st[b]["q"], st[b]["k"], st[b]["v"]
            O_ps = psO.tile([P, CT, 512], F32, tag="O")
            if Z_MODE == "pe":
                z_ps = psz.tile([1, 512], F32, tag="z")
                st[b]["zps%d" % qc] = z_ps
            else:
                zacc_g = rzpool.tile([P, 512], F32, tag="zaccg")
                zacc_v = rzpool.tile([P, 512], F32, tag="zaccv")
                st[b]["zacc%d" % qc] = (zacc_g, zacc_v)
            def s_matmul(nb):
                s_ps = psw.tile([P, 512], F32, tag="w", name="s_ps")
                for kc in range(CT):
                    nc.tensor.matmul(
                        s_ps,
                        lhsT=k_sb[:, kc, nb * P : (nb + 1) * P],
                        rhs=q_sb[:, kc, qc * 512 : (qc + 1) * 512],
                        start=(kc == 0),
                        stop=(kc == CT - 1),
                    )
                return s_ps

            # Two-deep software pipeline: S(nb+1), S(nb+2) are emitted
            # before the exp-gated z/O work of nb so the PE never waits
            # on ScalarE even when EXP runs slower than the S matmuls.
            s_fifo = [s_matmul(0), s_matmul(1)]
            for nb in range(NB):
                s_ps = s_fifo.pop(0)
                if nb + 2 < NB:
                    s_fifo.append(s_matmul(nb + 2))
                p_sb = ppool.tile([P, 512], BF16, tag="p")
                nc.scalar.activation(
                    out=p_sb, in_=s_ps, func=AF.Exp, bias=0.0, scale=1.0
                )
                # Z partial sums accumulate off the PE: the first half of
                # the k-blocks on GpSimd, the second half on DVE, so neither
                # chain straggles past the chunk's matmuls.
                if Z_MODE == "pe":
                    nc.tensor.matmul(
                        st[b]["zps%d" % qc],
                        lhsT=onesc_bf_sb,
                        rhs=p_sb,
                        start=(nb == 0),
                        stop=(nb == NB - 1),
                    )
                elif nb == 0:
                    nc.gpsimd.tensor_copy(out=zacc_g, in_=p_sb)
                elif nb < NB // 2:
                    nc.gpsimd.tensor_tensor(
                        out=zacc_g, in0=zacc_g, in1=p_sb, op=OP.add
                    )
                elif nb == NB // 2:
                    nc.vector.tensor_copy(out=zacc_v, in_=p_sb)
                else:
                    nc.vector.tensor_tensor(
                        out=zacc_v, in0=zacc_v, in1=p_sb, op=OP.add
                    )
                for ct in range(CT):
                    nc.tensor.matmul(
                        O_ps[:, ct, :],
                        lhsT=v_sb[:, nb, ct * P : (ct + 1) * P],
                        rhs=p_sb,
                        start=(nb == 0),
                        stop=(nb == NB - 1),
                    )
            # proj commutes with the per-column 1/Z scale, so proj depends
            # only on O: copy O out of PSUM here (releasing the O banks a
            # chunk early); the 1/Z bounce gates just the final DVE op.
            on_sb = opool.tile([P, CT, 512], BF16, tag="on")
            st[b]["on%d" % qc] = on_sb
            for ct in range(CT):
                nc.vector.tensor_copy(out=on_sb[:, ct, :], in_=O_ps[:, ct, :])

        def phase_rz(b, qc):
            if Z_MODE == "pe":
                z_ps = st[b].pop("zps%d" % qc)
            else:
                zacc_g, zacc_v = st[b].pop("zacc%d" % qc)
                nc.vector.tensor_add(out=zacc_v, in0=zacc_v, in1=zacc_g)
                z_ps = psz.tile([1, 512], F32, tag="z")
                nc.tensor.matmul(
                    z_ps, lhsT=onesc_sb, rhs=zacc_v, start=True, stop=True
                )
            # 1/Z with the row transposed to [128, 4] so the reciprocal
            # runs across lanes (a [1, 512] reciprocal costs ~4us on DVE).
            z_sb = rzpool.tile([1, 512], F32, tag="zsb")
            nc.vector.tensor_copy(out=z_sb, in_=z_ps)
            z_d = dram.tile([1, 512], F32, tag="zd")
            nc.sync.dma_start(out=z_d, in_=z_sb)
            zT_sb = rzpool.tile([P, 4], F32, tag="zT")
            nc.sync.dma_start(
                out=zT_sb, in_=z_d[0, :].rearrange("(p j) -> p j", j=4)
            )
            rzT_sb = rzpool.tile([P, 4], F32, tag="rzT")
            nc.vector.reciprocal(out=rzT_sb, in_=zT_sb)
            rz_d = dram.tile([1, 512], F32, tag="rzd")
            nc.sync.dma_start(
                out=rz_d[0, :].rearrange("(p j) -> p j", j=4), in_=rzT_sb
            )
            rzb_sb = rzpool.tile([P, 512], F32, tag="rzb")
            st[b]["rzb%d" % qc] = rzb_sb
            nc.sync.dma_start(out=rzb_sb, in_=rz_d[:, :].to_broadcast((P, 512)))

        def phase_d(b, qc):
            """Apply 1/Z, proj conv, residual add, store."""
            rzb_sb = st[b].pop("rzb%d" % qc)
            x_t = st[b]["x"]
            on_sb = st[b].pop("on%d" % qc)
            for ct in range(CT):
                pr_ps = psw.tile([P, 512], F32, tag="w")
                for kc in range(CT):
                    nc.tensor.matmul(
                        pr_ps,
                        lhsT=wp_sb[:, kc, ct * P : (ct + 1) * P],
                        rhs=on_sb[:, kc, :],
                        start=(kc == 0),
                        stop=(kc == CT - 1),
                    )
                o_sb = outp.tile([P, 512], F32, tag="o")
                xres = x_t[:, ct, qc * 512 : (qc + 1) * 512]
                nc.vector.tensor_mul(out=o_sb, in0=pr_ps, in1=rzb_sb)
                if use_bf:
                    nc.vector.scalar_tensor_tensor(
                        out=o_sb,
                        in0=o_sb,
                        scalar=bf_sb[:, ct : ct + 1],
                        in1=xres,
                        op0=OP.add,
                        op1=OP.add,
                    )
                else:
                    nc.vector.tensor_add(out=o_sb, in0=o_sb, in1=xres)
                nc.sync.dma_start(
                    out=out[b, ct * P : (ct + 1) * P, qc * 512 : (qc + 1) * 512],
                    in_=o_sb,
                )

        # Software pipeline: hide the stats chain of image b+1 under the
        # attention of image b, and each q-chunk's 1/Z DRAM bounce under
        # the next chunk's matmuls.
        phase_a(0)
        load_weights()
        for _ in range(N_WARM):
            warm_ps = psw.tile([P, 512], F32, tag="w", name="warm_ps")
            nc.tensor.matmul(
                warm_ps[:, 0:256], lhsT=wq_sb[:, 0, 0:P],
                rhs=wq_sb[:, 0, 0:256], start=True, stop=True,
            )
        pending = None
        for b in range(BL):
            phase_b(b)
            if b + 1 < BL:
                phase_a(b + 1)
            for qc in range(QCH):
                phase_c(b, qc)
                if RZ_SPLIT:
                    if pending is not None:
                        phase_d(*pending)
                    phase_rz(b, qc)
                else:
                    phase_rz(b, qc)
                    if pending is not None:
                        phase_d(*pending)
                pending = (b, qc)
        phase_d(*pending)
    nc.compile()
    return nc


def prepare(inputs):
    """Fold parameters on the host; return (program, per-core input maps)."""
    x = np.ascontiguousarray(np.asarray(inputs["x"], dtype=np.float32))
    norm_w = np.asarray(inputs["norm_w"], dtype=np.float32)
    norm_b = np.asarray(inputs["norm_b"], dtype=np.float32)
    qkv_w = np.asarray(inputs["qkv_w"], dtype=np.float32)
    qkv_b = np.asarray(inputs["qkv_b"], dtype=np.float32)
    proj_w = np.asarray(inputs["proj_w"], dtype=np.float32)
    proj_b = np.asarray(inputs["proj_b"], dtype=np.float32)

    # Fold the GroupNorm affine into qkv: qkv(h*w+b) = (qkv*w)h + qkv@b
    w_eff = qkv_w * norm_w[None, :]
    b_eff = qkv_b + qkv_w @ norm_b
    s4 = float(C) ** -0.25  # sqrt of the attention 1/sqrt(C) scale
    bf16 = ml_dtypes.bfloat16
    wq_t = np.ascontiguousarray((w_eff[0:C] * s4).T.astype(bf16))
    wk_t = np.ascontiguousarray((w_eff[C : 2 * C] * s4).T.astype(bf16))
    wv_t = np.ascontiguousarray(w_eff[2 * C : 3 * C].T.astype(bf16))
    wp_t = np.ascontiguousarray(proj_w.T.astype(bf16))
    bq_f = np.ascontiguousarray(b_eff[0:C] * s4)
    bk_f = np.ascontiguousarray(b_eff[C : 2 * C] * s4)
    bv_f = b_eff[2 * C : 3 * C]
    bf_f = np.ascontiguousarray(proj_w @ bv_f + proj_b)

    use_bq = bool(np.any(bq_f))
    use_bk = bool(np.any(bk_f))
    use_bf = bool(np.any(bf_f))
    nc = build_program(use_bq, use_bk, use_bf)

    xr = x.reshape(NCORES, BL, C, N)
    in_maps = []
    for c in range(NCORES):
        in_maps.append(
            {
                "xs": np.ascontiguousarray(xr[c]),
                "wq": wq_t,
                "wk": wk_t,
                "wv": wv_t,
                "wp": wp_t,
                "bq": bq_f,
                "bk": bk_f,
                "bf": bf_f,
            }
        )
    return nc, in_maps


def run(inputs, trace=False):
    from concourse.bass_utils import run_bass_kernel_spmd

    nc, in_maps = prepare(inputs)
    res = run_bass_kernel_spmd(nc, in_maps, list(range(NCORES)), trace=trace)
    outs = np.stack([np.asarray(res.results[i]["out"]) for i in range(NCORES)])
    full = outs.reshape(B, C, H, W).astype(np.float32)
    return full, res


def kernel(**inputs) -> np.ndarray:
    full, _ = run(inputs, trace=False)
    return full



# revision 11
# speedup vs baseline: 1.0625x; 1.0625x over previous
"""AttentionBlock Trainium2 kernel (Bass/Tile), SPMD over 8 NeuronCores.

Problem (hardcoded): x [32, 256, 32, 32] fp32
  GroupNorm(8 groups, eps=1e-5, affine) -> 1x1 qkv conv [768,256] ->
  per-image attention over N=1024 pixels (C=256) -> 1x1 proj [256,256] ->
  residual add.

Sharding: pure data-parallel over batch: 4 images per core, weights
replicated, no collectives.

v2 (fp8 DoubleRow rewrite of the bf16 baseline, ~184us):
  - All matmuls run in fp8e4 (max 240) with perf_mode=DoubleRow: the
    contraction dim doubles to 256, halving PE instruction count and
    streaming 2 elements/cell/cycle. Weights are pre-scaled by 8 on the
    host so their std ~0.5 sits in e4m3's healthy range; the scales
    cancel exactly: exp scale 1/1024 folds aq*ak*sqrt(C), the Z-matmul
    "ones" vector is 64 = av*ap, so proj output needs no rescale.
  - exp(S) would overflow fp8 (e^6.2 > 240), so a constant shift of 4
    is folded into the exp bias; softmax shift-invariance makes this
    exact.
  - x is cast to bf16 on the host (halves DMA + 2x DVE stats rate);
    output is stored bf16 and upcast on the host.
  - GroupNorm stats: per-channel sum/sumsq via tensor_reduce +
    tensor_tensor_reduce (accum_out), pooled to groups with a tiny
    matmul, rstd via bit-trick rsqrt + 1 Newton step on DVE (keeps the
    ACT table on Exp permanently - no table swaps), broadcast back to
    channels with a second tiny matmul (no DRAM bounce).
  - 1/Z: Z row [1,512] from a DoubleRow ones-matmul over P, copied to
    SBUF, broadcast across partitions on GpSimd (no DRAM bounce, no
    PSUM bank), reciprocal on DVE; 1/Z is fused into the O PSUM->SBUF
    cast so proj runs on normalized fp8 O directly.
  - Copies balanced across ACT (q, k, pcs) / DVE (v, z, recip, O-scale,
    final residual add) / GpSimd (h scale-shift, Z broadcast).
"""

from contextlib import ExitStack

import ml_dtypes
import numpy as np

import concourse.bass as bass
import concourse.tile as tile
from concourse import bacc
from concourse import mybir

F32 = mybir.dt.float32
BF16 = mybir.dt.bfloat16
F8 = mybir.dt.float8e4
U32 = mybir.dt.uint32
AF = mybir.ActivationFunctionType
OP = mybir.AluOpType
DR = mybir.MatmulPerfMode.DoubleRow
AX = mybir.AxisListType

B, C, H, W = 32, 256, 32, 32
N = H * W            # 1024
G = 8                # groups
EPS = 1e-5
NCORES = 8
BL = B // NCORES     # images per core
CT = C // 128        # channel tiles
NB = N // 128        # pixel blocks (k dim of attention)
QCH = N // 512       # 512-wide q chunks
NPAIR = NB // 2      # k-block pairs per chunk
P = 128
WSCALE = 8.0         # host-side fp8 weight scale (aq=ak=av=ap=8)
EXP_SHIFT = 4.0      # exp(S - shift); |S|<=~6.2 so P <= e^2.2 << 240
EXP_SCALE = 1.0 / (WSCALE * WSCALE * 16.0)  # aq*ak*sqrt(C) = 1024
ZONES = 64.0         # av*ap: folds the O/proj descale into 1/Z

import os as _os
N_WARM = int(_os.environ.get("KERNEL_N_WARM", "24"))
H_ON_GPSIMD = _os.environ.get("KERNEL_H_GPSIMD", "0") == "1"
VCOPY_ACT = int(_os.environ.get("KERNEL_VCOPY_ACT", "2"))  # of 4 v copies on ACT
ZB_MODE = _os.environ.get("KERNEL_ZB_MODE", "gpsimd")  # 'gpsimd' | 'dram'
USE_DR = _os.environ.get("KERNEL_DR", "1") == "1"


def build_program(use_bq: bool, use_bk: bool, use_bf: bool) -> bass.Bass:
    nc = bacc.Bacc()

    xs = nc.dram_tensor("xs", [BL, C, N], BF16, kind="ExternalInput")
    wq = nc.dram_tensor("wq", [C, C], F8, kind="ExternalInput")  # [c_in, c_out]
    wk = nc.dram_tensor("wk", [C, C], F8, kind="ExternalInput")
    wv = nc.dram_tensor("wv", [C, C], F8, kind="ExternalInput")
    wp = nc.dram_tensor("wp", [C, C], F8, kind="ExternalInput")
    bq = nc.dram_tensor("bq", [C], F32, kind="ExternalInput")
    bk = nc.dram_tensor("bk", [C], F32, kind="ExternalInput")
    bf = nc.dram_tensor("bf", [C], F32, kind="ExternalInput")
    out = nc.dram_tensor("out", [BL, C, N], BF16, kind="ExternalOutput")

    # Group-pool matmul: partition p -> group p//32, value 1/(32*1024)
    # (mean over the group's 32 channels x 1024 pixels; 2^-15 exact).
    gmask_np = np.zeros((P, 4), np.float32)
    gmask_np[np.arange(P), np.arange(P) // 32] = 1.0 / 32768.0
    gmask_d = nc.inline_tensor(gmask_np.astype(ml_dtypes.bfloat16), "gmask")
    # Broadcast matmul: group g -> partitions 32g..32g+31.
    sel_np = np.zeros((4, P), np.float32)
    sel_np[np.arange(P) // 32, np.arange(P)] = 1.0
    sel_d = nc.inline_tensor(sel_np.astype(ml_dtypes.bfloat16), "sel")

    with tile.TileContext(nc) as tc, ExitStack() as ctx:
        consts = ctx.enter_context(tc.tile_pool(name="consts", bufs=1))
        xpool = ctx.enter_context(tc.tile_pool(name="xp", bufs=2))
        hpool = ctx.enter_context(tc.tile_pool(name="hp", bufs=2))
        qpool = ctx.enter_context(tc.tile_pool(name="qp", bufs=2))
        kpool = ctx.enter_context(tc.tile_pool(name="kp", bufs=2))
        vpool = ctx.enter_context(tc.tile_pool(name="vp", bufs=2))
        ppool = ctx.enter_context(tc.tile_pool(name="pp", bufs=3))
        opool = ctx.enter_context(tc.tile_pool(name="op", bufs=2))
        spool = ctx.enter_context(tc.tile_pool(name="sp", bufs=2))
        rzpool = ctx.enter_context(tc.tile_pool(name="rzp", bufs=2))
        outp = ctx.enter_context(tc.tile_pool(name="outp", bufs=2))
        dram = ctx.enter_context(tc.tile_pool(name="dram", bufs=2, space="DRAM"))
        ps_s = ctx.enter_context(tc.tile_pool(name="pss", bufs=2, space="PSUM"))
        ps_O = ctx.enter_context(tc.tile_pool(name="psO", bufs=1, space="PSUM"))
        ps_z = ctx.enter_context(tc.tile_pool(name="psz", bufs=1, space="PSUM"))
        ps_pj = ctx.enter_context(tc.tile_pool(name="pspj", bufs=1, space="PSUM"))

        # --- constants ---
        gmask_sb = consts.tile([P, 4], BF16, tag="gmask")
        nc.sync.dma_start(out=gmask_sb, in_=gmask_d[:, :])
        sel_sb = consts.tile([4, P], BF16, tag="sel")
        nc.sync.dma_start(out=sel_sb, in_=sel_d[:, :])
        bq_sb = consts.tile([P, CT], F32, tag="bq")
        nc.sync.dma_start(out=bq_sb, in_=bq[:].rearrange("(t p) -> p t", p=P))
        bk_sb = consts.tile([P, CT], F32, tag="bk")
        nc.sync.dma_start(out=bk_sb, in_=bk[:].rearrange("(t p) -> p t", p=P))
        bf_sb = consts.tile([P, CT], F32, tag="bf")
        nc.sync.dma_start(out=bf_sb, in_=bf[:].rearrange("(t p) -> p t", p=P))
        ones64_sb = consts.tile([P, 2, 16], F8, tag="ones64")
        nc.vector.memset(ones64_sb, ZONES)
        magic_sb = consts.tile([4, CT], U32, tag="magic")
        nc.vector.memset(magic_sb, 0x5F3759DF)
        one_u32_sb = consts.tile([4, CT], U32, tag="oneu")
        nc.vector.memset(one_u32_sb, 1)
        eps_sb = consts.tile([4, 1], F32, tag="eps")
        nc.vector.memset(eps_sb, EPS)
        nshift_sb = consts.tile([P, 1], F32, tag="nshift")
        nc.vector.memset(nshift_sb, -EXP_SHIFT)
        wq_sb = consts.tile([P, CT, C], F8, tag="wq")
        wk_sb = consts.tile([P, CT, C], F8, tag="wk")
        wv_sb = consts.tile([P, CT, C], F8, tag="wv")
        wp_sb = consts.tile([P, CT, C], F8, tag="wp")

        def load_weights():
            for t_sb, t_d in ((wq_sb, wq), (wk_sb, wk), (wv_sb, wv), (wp_sb, wp)):
                nc.sync.dma_start(
                    out=t_sb, in_=t_d[:, :].rearrange("(t p) o -> p t o", p=P)
                )

        st = [dict() for _ in range(BL)]

        def mm2(out_ap, lhsT3, rhs3, start=True, stop=True):
            """One DoubleRow matmul (K=256), or two plain fp8 matmuls."""
            if USE_DR:
                nc.tensor.matmul(
                    out_ap, lhsT=lhsT3, rhs=rhs3, start=start, stop=stop,
                    perf_mode=DR,
                )
            else:
                for i in range(2):
                    nc.tensor.matmul(
                        out_ap,
                        lhsT=lhsT3[:, i, :],
                        rhs=rhs3[:, i, :],
                        start=(start and i == 0),
                        stop=(stop and i == 1),
                    )

        def load_x(b):
            x_t = xpool.tile([P, CT, N], BF16, tag="x")
            st[b]["x"] = x_t
            nc.sync.dma_start(
                out=x_t, in_=xs[b].rearrange("(t p) n -> p t n", p=P)
            )

        def phase_stats(b):
            """GroupNorm stats -> per-channel (mean, rstd) -> h (fp8)."""
            x_t = st[b]["x"]
            cs = spool.tile([P, CT, 2], F32, tag="cs")
            scr = spool.tile([P, N], BF16, tag="scr")
            for ct in range(CT):
                nc.vector.tensor_reduce(
                    out=cs[:, ct, 0:1], in_=x_t[:, ct, :], axis=AX.X, op=OP.add
                )
                nc.vector.tensor_mul(out=scr, in0=x_t[:, ct, :], in1=x_t[:, ct, :])
                nc.vector.tensor_reduce(
                    out=cs[:, ct, 1:2], in_=scr, axis=AX.X, op=OP.add
                )
            cs_bf = spool.tile([P, CT, 2], BF16, tag="csbf")
            nc.vector.tensor_copy(out=cs_bf, in_=cs)
            gm_ps = ps_z.tile([4, CT, 2], F32, tag="z", name="gm_ps")
            nc.tensor.matmul(
                gm_ps.rearrange("p a b -> p (a b)"),
                lhsT=gmask_sb,
                rhs=cs_bf.rearrange("p a b -> p (a b)"),
                start=True,
                stop=True,
            )
            gsb = spool.tile([4, CT, 2], F32, tag="gsb")
            nc.scalar.copy(out=gsb, in_=gm_ps)
            # gvar = E[x^2] - E[x]^2  (per group, per ct)
            gmean = gsb[:, :, 0]
            ge2 = gsb[:, :, 1]
            msq = spool.tile([4, CT], F32, tag="msq")
            nc.vector.tensor_mul(out=msq, in0=gmean, in1=gmean)
            gvar = spool.tile([4, CT], F32, tag="gvar")
            nc.vector.tensor_tensor(out=gvar, in0=ge2, in1=msq, op=OP.subtract)
            # rstd = rsqrt(gvar + eps): bit-trick seed + 1 Newton step.
            yu = spool.tile([4, CT], U32, tag="yu")
            nc.vector.tensor_tensor(
                out=yu,
                in0=gvar.bitcast(U32),
                in1=one_u32_sb,
                op=OP.logical_shift_right,
            )
            nc.vector.tensor_tensor(
                out=yu, in0=magic_sb, in1=yu, op=OP.subtract
            )
            y = yu.bitcast(F32)
            y2 = spool.tile([4, CT], F32, tag="y2")
            nc.vector.tensor_mul(out=y2, in0=y, in1=y)
            nc.vector.scalar_tensor_tensor(
                out=y2, in0=gvar, scalar=eps_sb, in1=y2,
                op0=OP.add, op1=OP.mult,
            )
            nc.vector.tensor_scalar(
                out=y2, in0=y2, scalar1=-0.5, scalar2=1.5,
                op0=OP.mult, op1=OP.add,
            )
            grstd = spool.tile([4, CT], F32, tag="grstd")
            nc.vector.tensor_mul(out=grstd, in0=y, in1=y2)
            gfin = spool.tile([4, CT, 2], BF16, tag="gfin")
            nc.vector.tensor_copy(out=gfin[:, :, 0], in_=gmean)
            nc.vector.tensor_copy(out=gfin[:, :, 1], in_=grstd)
            pcs_ps = ps_z.tile([P, CT, 2], F32, tag="z", name="pcs_ps")
            nc.tensor.matmul(
                pcs_ps.rearrange("p a b -> p (a b)"),
                lhsT=sel_sb,
                rhs=gfin.rearrange("p a b -> p (a b)"),
                start=True,
                stop=True,
            )
            pcs = spool.tile([P, CT, 2], F32, tag="pcs")
            nc.scalar.copy(out=pcs, in_=pcs_ps)
            h_t = hpool.tile([P, CT, N], F8, tag="h")
            st[b]["h"] = h_t
            eng = nc.gpsimd if H_ON_GPSIMD else nc.vector
            for ct in range(CT):
                eng.tensor_scalar(
                    out=h_t[:, ct, :],
                    in0=x_t[:, ct, :],
                    scalar1=pcs[:, ct, 0:1],
                    scalar2=pcs[:, ct, 1:2],
                    op0=OP.subtract,
                    op1=OP.mult,
                )

        def phase_b(b):
            """qkv 1x1 convs (fp8 DoubleRow, contraction 256 per MM)."""
            h_t = st[b]["h"]
            q_sb = qpool.tile([P, CT, N], F8, tag="q")
            k_sb = kpool.tile([P, CT, N], F8, tag="k")
            st[b]["q"], st[b]["k"] = q_sb, k_sb
            for dst, w_sb, b_sb, use_b in (
                (q_sb, wq_sb, bq_sb, use_bq),
                (k_sb, wk_sb, bk_sb, use_bk),
            ):
                for ct in range(CT):
                    mm = ps_s.tile([P, 2, 512], F32, tag="s", name="qk_ps")
                    for nch in range(2):
                        mm2(
                            mm[:, nch, :],
                            w_sb[:, 0:2, ct * P : (ct + 1) * P],
                            h_t[:, 0:2, nch * 512 : (nch + 1) * 512],
                        )
                    nc.scalar.activation(
                        out=dst[:, ct, :].rearrange("p (a b) -> p a b", a=2),
                        in_=mm,
                        func=AF.Identity,
                        bias=b_sb[:, ct : ct + 1] if use_b else 0.0,
                        scale=1.0,
                    )
            v_sb = vpool.tile([P, NB, C], F8, tag="v")
            st[b]["v"] = v_sb
            for np_ in range(NPAIR):
                vv = ps_s.tile([P, 2, 512], F32, tag="s", name="v_ps")
                for i in range(2):
                    nb = 2 * np_ + i
                    mm2(
                        vv[:, i, 0:C],
                        h_t[:, 0:2, nb * P : (nb + 1) * P],
                        wv_sb[:, 0:2, :],
                    )
                if np_ < VCOPY_ACT:
                    nc.scalar.copy(
                        out=v_sb[:, 2 * np_ : 2 * np_ + 2, :], in_=vv[:, :, 0:C]
                    )
                else:
                    nc.vector.tensor_copy(
                        out=v_sb[:, 2 * np_ : 2 * np_ + 2, :], in_=vv[:, :, 0:C]
                    )

        def phase_c(b, qc, pending):
            """Attention chunk: S^T pairs, exp, Z, O, then the 1/Z chain."""
            q_sb, k_sb, v_sb = st[b]["q"], st[b]["k"], st[b]["v"]
            O_ps = ps_O.tile([P, CT, 512], F32, tag="O")
            z_ps = ps_z.tile([1, 512], F32, tag="z", name="z_ps")

            def s_pair(j):
                s2 = ps_s.tile([P, 2, 512], F32, tag="s", name="s2_ps")
                for i in range(2):
                    nb = 2 * j + i
                    mm2(
                        s2[:, i, :],
                        k_sb[:, 0:2, nb * P : (nb + 1) * P],
                        q_sb[:, 0:2, qc * 512 : (qc + 1) * 512],
                    )
                return s2

            fifo = [s_pair(0), s_pair(1)]
            if pending is not None:
                phase_d(*pending)
            if qc == 0 and b + 1 < BL:
                load_x(b + 1)
            for j in range(NPAIR):
                s2 = fifo.pop(0)
                if j + 2 < NPAIR:
                    fifo.append(s_pair(j + 2))
                p2 = ppool.tile([P, 2, 512], F8, tag="p")
                if j == 0:
                    # split the first exp so Z/O start sooner
                    for i in range(2):
                        nc.scalar.activation(
                            out=p2[:, i, :], in_=s2[:, i, :], func=AF.Exp,
                            bias=nshift_sb, scale=EXP_SCALE,
                        )
                else:
                    nc.scalar.activation(
                        out=p2, in_=s2, func=AF.Exp,
                        bias=nshift_sb, scale=EXP_SCALE,
                    )
                mm2(
                    z_ps,
                    ones64_sb[:, :, 0:1],
                    p2,
                    start=(j == 0),
                    stop=(j == NPAIR - 1),
                )
                for ct in range(CT):
                    mm2(
                        O_ps[:, ct, :],
                        v_sb[:, 2 * j : 2 * j + 2, ct * P : (ct + 1) * P],
                        p2,
                        start=(j == 0),
                        stop=(j == NPAIR - 1),
                    )
            # 1/Z chain, all on-chip: PSUM -> SBUF -> gpsimd partition
            # broadcast -> DVE reciprocal -> fused into the O cast.
            z_sb = rzpool.tile([1, 512], F32, tag="zsb")
            nc.vector.tensor_copy(out=z_sb, in_=z_ps)
            zb = rzpool.tile([P, 512], F32, tag="zb")
            if ZB_MODE == "gpsimd":
                nc.gpsimd.partition_broadcast(zb, z_sb)
            else:
                z_d = dram.tile([1, 512], F32, tag="zd")
                nc.sync.dma_start(out=z_d, in_=z_sb)
                nc.sync.dma_start(out=zb, in_=z_d[:, :].to_broadcast((P, 512)))
            rz = rzpool.tile([P, 512], F32, tag="rz")
            nc.vector.reciprocal_approx_fast(out=rz, in_=zb)
            on_sb = opool.tile([P, CT, 512], F8, tag="on")
            st[b]["on%d" % qc] = on_sb
            for ct in range(CT):
                nc.vector.tensor_mul(
                    out=on_sb[:, ct, :], in0=O_ps[:, ct, :], in1=rz
                )

        def phase_d(b, qc):
            """proj conv on normalized O, residual add, store."""
            x_t = st[b]["x"]
            on_sb = st[b].pop("on%d" % qc)
            o_sb = outp.tile([P, CT, 512], BF16, tag="o")
            for ct in range(CT):
                pj = ps_pj.tile([P, 512], F32, tag="pj")
                mm2(
                    pj,
                    wp_sb[:, 0:2, ct * P : (ct + 1) * P],
                    on_sb,
                )
                xres = x_t[:, ct, qc * 512 : (qc + 1) * 512]
                if use_bf:
                    nc.vector.scalar_tensor_tensor(
                        out=o_sb[:, ct, :],
                        in0=pj,
                        scalar=bf_sb[:, ct : ct + 1],
                        in1=xres,
                        op0=OP.add,
                        op1=OP.add,
                    )
                else:
                    nc.vector.tensor_add(out=o_sb[:, ct, :], in0=pj, in1=xres)
            nc.sync.dma_start(
                out=out[b, :, qc * 512 : (qc + 1) * 512].rearrange(
                    "(t p) n -> p t n", p=P
                ),
                in_=o_sb,
            )

        # --- emission schedule ---
        load_x(0)
        load_weights()
        for _ in range(N_WARM):
            warm = ps_s.tile([P, 2, 512], F32, tag="s", name="warm_ps")
            nc.tensor.matmul(
                warm[:, 0, 0:256], lhsT=wq_sb[:, 0, 0:P],
                rhs=wq_sb[:, 0, 0:256], start=True, stop=True,
            )
        phase_stats(0)
        phase_b(0)
        pending = None
        for b in range(BL):
            for qc in range(QCH):
                phase_c(b, qc, pending)
                pending = (b, qc)
                if qc == 0 and b + 1 < BL:
                    phase_stats(b + 1)
                if qc == 1 and b + 1 < BL:
                    phase_b(b + 1)
        phase_d(*pending)
    nc.compile()
    return nc


def prepare(inputs):
    """Fold parameters on the host; return (program, per-core input maps)."""
    x = np.asarray(inputs["x"], dtype=np.float32)
    norm_w = np.asarray(inputs["norm_w"], dtype=np.float32)
    norm_b = np.asarray(inputs["norm_b"], dtype=np.float32)
    qkv_w = np.asarray(inputs["qkv_w"], dtype=np.float32)
    qkv_b = np.asarray(inputs["qkv_b"], dtype=np.float32)
    proj_w = np.asarray(inputs["proj_w"], dtype=np.float32)
    proj_b = np.asarray(inputs["proj_b"], dtype=np.float32)

    # Fold the GroupNorm affine into qkv: qkv(h*w+b) = (qkv*w)h + qkv@b
    w_eff = qkv_w * norm_w[None, :]
    b_eff = qkv_b + qkv_w @ norm_b
    f8 = ml_dtypes.float8_e4m3
    bf16 = ml_dtypes.bfloat16
    wq_t = np.ascontiguousarray((w_eff[0:C] * WSCALE).T.astype(f8))
    wk_t = np.ascontiguousarray((w_eff[C : 2 * C] * WSCALE).T.astype(f8))
    wv_t = np.ascontiguousarray((w_eff[2 * C : 3 * C] * WSCALE).T.astype(f8))
    wp_t = np.ascontiguousarray((proj_w * WSCALE).T.astype(f8))
    bq_f = np.ascontiguousarray(b_eff[0:C] * WSCALE)
    bk_f = np.ascontiguousarray(b_eff[C : 2 * C] * WSCALE)
    bv_f = b_eff[2 * C : 3 * C]
    bf_f = np.ascontiguousarray(proj_w @ bv_f + proj_b)

    use_bq = bool(np.any(bq_f))
    use_bk = bool(np.any(bk_f))
    use_bf = bool(np.any(bf_f))
    nc = build_program(use_bq, use_bk, use_bf)

    xr = x.reshape(NCORES, BL, C, N).astype(bf16)
    in_maps = []
    for c in range(NCORES):
        in_maps.append(
            {
                "xs": np.ascontiguousarray(xr[c]),
                "wq": wq_t,
                "wk": wk_t,
                "wv": wv_t,
                "wp": wp_t,
                "bq": bq_f,
                "bk": bk_f,
                "bf": bf_f,
            }
        )
    return nc, in_maps


def run(inputs, trace=False):
    from concourse.bass_utils import run_bass_kernel_spmd

    nc, in_maps = prepare(inputs)
    res = run_bass_kernel_spmd(nc, in_maps, list(range(NCORES)), trace=trace)
    outs = np.stack(
        [np.asarray(res.results[i]["out"]) for i in range(NCORES)]
    )
    full = outs.reshape(B, C, H, W).astype(np.float32)
    return full, res


def kernel(**inputs) -> np.ndarray:
    full, _ = run(inputs, trace=False)
    return full


# revision 13
# speedup vs baseline: 1.4983x; 1.4102x over previous
"""AttentionBlock Trainium2 kernel (Bass/Tile), SPMD over 8 NeuronCores.

Problem (hardcoded): x [32, 256, 32, 32] fp32
  GroupNorm(8 groups, eps=1e-5, affine) -> 1x1 qkv conv [768,256] ->
  per-image attention over N=1024 pixels (C=256) -> 1x1 proj [256,256] ->
  residual add.

Sharding: pure data-parallel over batch: 4 images per core, weights
replicated, no collectives.

v3 (fp8 DoubleRow, ~2x over the bf16 baseline):
  - All matmuls fp8e4 with perf_mode=DoubleRow (K=256 per instruction,
    2 fp8 MACs/cell/cycle -> 512 cycles for a [256x128]x[256x512] MM).
  - S = h^T (Wk^T Wq) h: the q/k convs fold into ONE conv with the
    host-precomputed M = 32*Wk^T Wq (fp8-healthy std ~2), eliminating
    one conv and two PSUM->SBUF copy passes. exp scale 1/512 undoes it.
    (Falls back to separate q/k convs when qkv biases are nonzero.)
  - exp(S-4) keeps P in e4m3 range (shift cancels in softmax); the
    Z-matmul ones-vector is 64 = av*ap so proj needs no rescale.
  - x bf16 (host-cast), out bf16 (host-upcast).
  - GroupNorm: bn_stats/bn_aggr, group-pool + channel-broadcast via two
    tiny matmuls (PSUM slots borrowed from the proj bank), rstd via
    bit-trick rsqrt + 1 Newton step on DVE (no ACT table swaps, the ACT
    table stays on Exp).
  - 1/Z: DoubleRow ones-matmul -> ACT copy to SBUF -> GpSimd partition
    broadcast -> DVE reciprocal_approx_fast, fused into the O cast.
  - Next image's stats emit inside chunk 0, its convs interleave with
    chunk 1's attention pairs (shared PSUM ring alternates pair/conv
    tiles so neither stream starves).
"""

from contextlib import ExitStack

import ml_dtypes
import numpy as np

import concourse.bass as bass
import concourse.tile as tile
from concourse import bacc
from concourse import mybir

F32 = mybir.dt.float32
BF16 = mybir.dt.bfloat16
F8 = mybir.dt.float8e4
U32 = mybir.dt.uint32
AF = mybir.ActivationFunctionType
OP = mybir.AluOpType
DR = mybir.MatmulPerfMode.DoubleRow
AX = mybir.AxisListType

B, C, H, W = 32, 256, 32, 32
N = H * W            # 1024
G = 8                # groups
EPS = 1e-5
NCORES = 8
BL = B // NCORES     # images per core
CT = C // 128        # channel tiles
NB = N // 128        # pixel blocks (k dim of attention)
QCH = N // 512       # 512-wide q chunks
NPAIR = NB // 2      # k-block pairs per chunk
P = 128
WSCALE = 8.0         # host fp8 scale for wv / wp
MSCALE = 32.0        # host fp8 scale for M = Wk^T Wq
EXP_SHIFT = 4.0      # exp(S - shift); |S| <= ~6.2
ZONES = 64.0         # av*ap: folds the O/proj descale into 1/Z

import os as _os
N_WARM = int(_os.environ.get("KERNEL_N_WARM", "40"))
VCOPY_ACT = int(_os.environ.get("KERNEL_VCOPY_ACT", "2"))  # of 4 v copies on ACT


def build_program(use_bq: bool, use_bk: bool, use_bf: bool) -> bass.Bass:
    use_qk = use_bq or use_bk  # fallback: separate q/k convs with biases
    exp_scale = 1.0 / (16.0 * (WSCALE * WSCALE if use_qk else MSCALE))

    nc = bacc.Bacc()

    xs = nc.dram_tensor("xs", [BL, C, N], BF16, kind="ExternalInput")
    wq = nc.dram_tensor("wq", [C, C], F8, kind="ExternalInput")  # [c_in, c_out]
    wk = nc.dram_tensor("wk", [C, C], F8, kind="ExternalInput")
    wv = nc.dram_tensor("wv", [C, C], F8, kind="ExternalInput")
    wp = nc.dram_tensor("wp", [C, C], F8, kind="ExternalInput")
    bq = nc.dram_tensor("bq", [C], F32, kind="ExternalInput")
    bk = nc.dram_tensor("bk", [C], F32, kind="ExternalInput")
    bf = nc.dram_tensor("bf", [C], F32, kind="ExternalInput")
    out = nc.dram_tensor("out", [BL, C, N], BF16, kind="ExternalOutput")

    # Group-pool matmul: partition p -> group p//32; bn_aggr already
    # yields per-channel means, so pooling averages 32 channels (1/32).
    gmask_np = np.zeros((P, 4), np.float32)
    gmask_np[np.arange(P), np.arange(P) // 32] = 1.0 / 32.0
    gmask_d = nc.inline_tensor(gmask_np.astype(ml_dtypes.bfloat16), "gmask")
    # Broadcast matmul: group g -> partitions 32g..32g+31.
    sel_np = np.zeros((4, P), np.float32)
    sel_np[np.arange(P) // 32, np.arange(P)] = 1.0
    sel_d = nc.inline_tensor(sel_np.astype(ml_dtypes.bfloat16), "sel")

    with tile.TileContext(nc) as tc, ExitStack() as ctx:
        consts = ctx.enter_context(tc.tile_pool(name="consts", bufs=1))
        xpool = ctx.enter_context(tc.tile_pool(name="xp", bufs=2))
        hpool = ctx.enter_context(tc.tile_pool(name="hp", bufs=2))
        tpool = ctx.enter_context(tc.tile_pool(name="tp", bufs=2))
        vpool = ctx.enter_context(tc.tile_pool(name="vp", bufs=2))
        ppool = ctx.enter_context(tc.tile_pool(name="pp", bufs=3))
        opool = ctx.enter_context(tc.tile_pool(name="op", bufs=2))
        spool = ctx.enter_context(tc.tile_pool(name="sp", bufs=2))
        rzpool = ctx.enter_context(tc.tile_pool(name="rzp", bufs=2))
        outp = ctx.enter_context(tc.tile_pool(name="outp", bufs=2))
        ps_s = ctx.enter_context(tc.tile_pool(name="pss", bufs=2, space="PSUM"))
        ps_O = ctx.enter_context(tc.tile_pool(name="psO", bufs=1, space="PSUM"))
        ps_z = ctx.enter_context(tc.tile_pool(name="psz", bufs=1, space="PSUM"))
        ps_pj = ctx.enter_context(tc.tile_pool(name="pspj", bufs=1, space="PSUM"))

        # --- constants ---
        wdum_sb = consts.tile([P, 512], BF16, tag="wdum")
        nc.vector.memset(wdum_sb, 0.25)
        gmask_sb = consts.tile([P, 4], BF16, tag="gmask")
        nc.sync.dma_start(out=gmask_sb, in_=gmask_d[:, :])
        sel_sb = consts.tile([4, P], BF16, tag="sel")
        nc.sync.dma_start(out=sel_sb, in_=sel_d[:, :])
        bq_sb = consts.tile([P, CT], F32, tag="bq")
        nc.sync.dma_start(out=bq_sb, in_=bq[:].rearrange("(t p) -> p t", p=P))
        bk_sb = consts.tile([P, CT], F32, tag="bk")
        nc.sync.dma_start(out=bk_sb, in_=bk[:].rearrange("(t p) -> p t", p=P))
        bf_sb = consts.tile([P, CT], F32, tag="bf")
        nc.sync.dma_start(out=bf_sb, in_=bf[:].rearrange("(t p) -> p t", p=P))
        ones64_sb = consts.tile([P, 2, 16], F8, tag="ones64")
        nc.vector.memset(ones64_sb, ZONES)
        magic_sb = consts.tile([4, CT], U32, tag="magic")
        nc.vector.memset(magic_sb, 0x5F3759DF)
        one_u32_sb = consts.tile([4, CT], U32, tag="oneu")
        nc.vector.memset(one_u32_sb, 1)
        eps_sb = consts.tile([4, 1], F32, tag="eps")
        nc.vector.memset(eps_sb, EPS)
        nshift_sb = consts.tile([P, 1], F32, tag="nshift")
        nc.vector.memset(nshift_sb, -EXP_SHIFT)
        wq_sb = consts.tile([P, CT, C], F8, tag="wq")
        wk_sb = consts.tile([P, CT, C], F8, tag="wk") if use_qk else None
        wv_sb = consts.tile([P, CT, C], F8, tag="wv")
        wp_sb = consts.tile([P, CT, C], F8, tag="wp")

        def load_weights():
            pairs = [(wq_sb, wq), (wv_sb, wv), (wp_sb, wp)]
            if use_qk:
                pairs.append((wk_sb, wk))
            for t_sb, t_d in pairs:
                nc.sync.dma_start(
                    out=t_sb, in_=t_d[:, :].rearrange("(t p) o -> p t o", p=P)
                )

        st = [dict() for _ in range(BL)]

        def load_x(b, split=False):
            x_t = xpool.tile([P, CT, N], BF16, tag="x")
            st[b]["x"] = x_t
            if split:
                for ct in range(CT):
                    nc.sync.dma_start(
                        out=x_t[:, ct, :], in_=xs[b, ct * P : (ct + 1) * P, :]
                    )
            else:
                nc.sync.dma_start(
                    out=x_t, in_=xs[b].rearrange("(t p) n -> p t n", p=P)
                )

        def phase_stats(b):
            """GroupNorm stats -> per-channel (mean, rstd) -> h (fp8)."""
            x_t = st[b]["x"]
            cs = spool.tile([P, CT, 2], F32, tag="cs")
            cs_bf = spool.tile([P, CT, 2], BF16, tag="csbf")
            gm_ps = ps_pj.tile([4, CT, 2], F32, tag="pj", name="gm_ps")
            for ct in range(CT):
                bnst = spool.tile([P, 2, 6], F32, tag="bnst")
                for s in range(2):
                    nc.vector.bn_stats(
                        out=bnst[:, s, :], in_=x_t[:, ct, s * 512 : (s + 1) * 512]
                    )
                nc.vector.bn_aggr(out=cs[:, ct, :], in_=bnst)
            # E[x^2] = var + mean^2 (both cts in two strided ops)
            msq = spool.tile([P, CT], F32, tag="msq")
            nc.vector.tensor_mul(out=msq, in0=cs[:, :, 0], in1=cs[:, :, 0])
            nc.vector.tensor_tensor(
                out=cs[:, :, 1], in0=cs[:, :, 1], in1=msq, op=OP.add
            )
            nc.vector.tensor_copy(out=cs_bf, in_=cs)
            nc.tensor.matmul(
                gm_ps.rearrange("p a b -> p (a b)"),
                lhsT=gmask_sb,
                rhs=cs_bf.rearrange("p a b -> p (a b)"),
                start=True,
                stop=True,
            )
            gsb = spool.tile([4, CT, 2], F32, tag="gsb")
            nc.scalar.copy(out=gsb, in_=gm_ps)
            gmean = gsb[:, :, 0]
            ge2 = gsb[:, :, 1]
            msq4 = spool.tile([4, CT], F32, tag="msq4")
            nc.vector.tensor_mul(out=msq4, in0=gmean, in1=gmean)
            gvar = spool.tile([4, CT], F32, tag="gvar")
            nc.vector.tensor_tensor(out=gvar, in0=ge2, in1=msq4, op=OP.subtract)
            # rstd = rsqrt(gvar + eps): bit-trick seed + 1 Newton step.
            yu = spool.tile([4, CT], U32, tag="yu")
            nc.vector.tensor_tensor(
                out=yu, in0=gvar.bitcast(U32), in1=one_u32_sb,
                op=OP.logical_shift_right,
            )
            nc.vector.tensor_tensor(out=yu, in0=magic_sb, in1=yu, op=OP.subtract)
            y = yu.bitcast(F32)
            y2 = spool.tile([4, CT], F32, tag="y2")
            nc.vector.tensor_mul(out=y2, in0=y, in1=y)
            nc.vector.scalar_tensor_tensor(
                out=y2, in0=gvar, scalar=eps_sb, in1=y2, op0=OP.add, op1=OP.mult
            )
            nc.vector.tensor_scalar(
                out=y2, in0=y2, scalar1=-0.5, scalar2=1.5, op0=OP.mult, op1=OP.add
            )
            grstd = spool.tile([4, CT], F32, tag="grstd")
            nc.vector.tensor_mul(out=grstd, in0=y, in1=y2)
            gfin = spool.tile([4, CT, 2], BF16, tag="gfin")
            nc.vector.tensor_copy(out=gfin[:, :, 0], in_=gmean)
            nc.vector.tensor_copy(out=gfin[:, :, 1], in_=grstd)
            pcs_ps = ps_pj.tile([P, CT, 2], F32, tag="pj", name="pcs_ps")
            nc.tensor.matmul(
                pcs_ps.rearrange("p a b -> p (a b)"),
                lhsT=sel_sb,
                rhs=gfin.rearrange("p a b -> p (a b)"),
                start=True,
                stop=True,
            )
            pcs = spool.tile([P, CT, 2], F32, tag="pcs")
            nc.scalar.copy(out=pcs, in_=pcs_ps)
            h_t = hpool.tile([P, CT, N], F8, tag="h")
            st[b]["h"] = h_t
            for ct in range(CT):
                nc.vector.tensor_scalar(
                    out=h_t[:, ct, :],
                    in0=x_t[:, ct, :],
                    scalar1=pcs[:, ct, 0:1],
                    scalar2=pcs[:, ct, 1:2],
                    op0=OP.subtract,
                    op1=OP.mult,
                )

        def conv_pieces(b):
            """Psum-tile-granular conv work for image b: t (or q/k) + v."""
            h_t = st[b]["h"]
            t_sb = tpool.tile([P, CT, N], F8, tag="t")
            st[b]["t"] = t_sb
            if use_qk:
                k_sb = tpool.tile([P, CT, N], F8, tag="k")
                st[b]["k"] = k_sb
            pieces = []
            convs = [(t_sb, wq_sb, bq_sb, use_bq)]
            if use_qk:
                convs.append((st[b]["k"], wk_sb, bk_sb, use_bk))
            for dst, w_sb, b_sb, use_b in convs:
                for ct in range(CT):
                    def piece(dst=dst, w_sb=w_sb, b_sb=b_sb, use_b=use_b, ct=ct):
                        mm = ps_s.tile([P, 2, 512], F32, tag="s", name="tk_ps")
                        for nch in range(2):
                            nc.tensor.matmul(
                                mm[:, nch, :],
                                lhsT=w_sb[:, 0:2, ct * P : (ct + 1) * P],
                                rhs=h_t[:, 0:2, nch * 512 : (nch + 1) * 512],
                                start=True,
                                stop=True,
                                perf_mode=DR,
                            )
                        nc.scalar.activation(
                            out=dst[:, ct, :].rearrange("p (a b) -> p a b", a=2),
                            in_=mm,
                            func=AF.Identity,
                            bias=b_sb[:, ct : ct + 1] if use_b else 0.0,
                            scale=1.0,
                        )
                    pieces.append(piece)
            v_sb = vpool.tile([P, NB, C], F8, tag="v")
            st[b]["v"] = v_sb
            for np_ in range(NPAIR):
                def piece(np_=np_):
                    vv = ps_s.tile([P, 2, 512], F32, tag="s", name="v_ps")
                    for i in range(2):
                        nb = 2 * np_ + i
                        nc.tensor.matmul(
                            vv[:, i, 0:C],
                            lhsT=h_t[:, 0:2, nb * P : (nb + 1) * P],
                            rhs=wv_sb[:, 0:2, :],
                            start=True,
                            stop=True,
                            perf_mode=DR,
                        )
                    if np_ < VCOPY_ACT:
                        nc.scalar.copy(
                            out=v_sb[:, 2 * np_ : 2 * np_ + 2, :],
                            in_=vv[:, :, 0:C],
                        )
                    else:
                        nc.vector.tensor_copy(
                            out=v_sb[:, 2 * np_ : 2 * np_ + 2, :],
                            in_=vv[:, :, 0:C],
                        )
                pieces.append(piece)
            return pieces

        def phase_c(b, qc, pending, side):
            """Attention chunk: S^T pairs, exp, Z, O, then the 1/Z chain;
            `side` callables interleave one per pair iteration."""
            v_sb = st[b]["v"]
            h_t = st[b]["h"]
            t_sb = st[b]["t"]
            s_rhs = t_sb if use_qk else h_t  # q-side operand
            O_ps = ps_O.tile([P, CT, 512], F32, tag="O")
            z_ps = ps_z.tile([1, 512], F32, tag="z", name="z_ps")

            def s_pair(j):
                s2 = ps_s.tile([P, 2, 512], F32, tag="s", name="s2_ps")
                for i in range(2):
                    nb = 2 * j + i
                    lhs = st[b]["k"] if use_qk else t_sb
                    nc.tensor.matmul(
                        s2[:, i, :],
                        lhsT=lhs[:, 0:2, nb * P : (nb + 1) * P],
                        rhs=s_rhs[:, 0:2, qc * 512 : (qc + 1) * 512],
                        start=True,
                        stop=True,
                        perf_mode=DR,
                    )
                return s2

            fifo = [s_pair(0), s_pair(1)]
            if pending is not None:
                phase_d(*pending)
            if qc == 0 and b + 1 < BL:
                load_x(b + 1)
            side = list(side)
            for j in range(NPAIR):
                if j + 2 < NPAIR:
                    fifo.append(s_pair(j + 2))
                if j >= 1 and side:
                    side.pop(0)()
                s2 = fifo.pop(0)
                p2 = ppool.tile([P, 2, 512], F8, tag="p")
                if j == 0:
                    # split the first exp so Z/O start sooner
                    for i in range(2):
                        nc.scalar.activation(
                            out=p2[:, i, :], in_=s2[:, i, :], func=AF.Exp,
                            bias=nshift_sb, scale=exp_scale,
                        )
                else:
                    nc.scalar.activation(
                        out=p2, in_=s2, func=AF.Exp,
                        bias=nshift_sb, scale=exp_scale,
                    )
                nc.tensor.matmul(
                    z_ps,
                    lhsT=ones64_sb[:, :, 0:1],
                    rhs=p2,
                    start=(j == 0),
                    stop=(j == NPAIR - 1),
                    perf_mode=DR,
                )
                for ct in range(CT):
                    nc.tensor.matmul(
                        O_ps[:, ct, :],
                        lhsT=v_sb[:, 2 * j : 2 * j + 2, ct * P : (ct + 1) * P],
                        rhs=p2,
                        start=(j == 0),
                        stop=(j == NPAIR - 1),
                        perf_mode=DR,
                    )
            for fn in side:
                fn()
            # 1/Z chain, all on-chip: PSUM -> SBUF (ACT) -> GpSimd partition
            # broadcast -> DVE approx reciprocal -> fused into the O cast.
            z_sb = rzpool.tile([1, 512], F32, tag="zsb")
            nc.scalar.copy(out=z_sb, in_=z_ps)
            zb = rzpool.tile([P, 512], F32, tag="zb")
            nc.gpsimd.partition_broadcast(zb, z_sb)
            rz = rzpool.tile([P, 512], F32, tag="rz")
            nc.vector.reciprocal_approx_fast(out=rz, in_=zb)
            on_sb = opool.tile([P, CT, 512], F8, tag="on")
            st[b]["on%d" % qc] = on_sb
            for ct in range(CT):
                nc.vector.tensor_mul(
                    out=on_sb[:, ct, :], in0=O_ps[:, ct, :], in1=rz
                )

        def phase_d(b, qc):
            """proj conv on normalized O, residual add, store."""
            x_t = st[b]["x"]
            on_sb = st[b].pop("on%d" % qc)
            o_sb = outp.tile([P, CT, 512], BF16, tag="o")
            for ct in range(CT):
                pj = ps_pj.tile([P, 512], F32, tag="pj", name="pj_ps")
                nc.tensor.matmul(
                    pj,
                    lhsT=wp_sb[:, 0:2, ct * P : (ct + 1) * P],
                    rhs=on_sb,
                    start=True,
                    stop=True,
                    perf_mode=DR,
                )
                xres = x_t[:, ct, qc * 512 : (qc + 1) * 512]
                if use_bf:
                    nc.vector.scalar_tensor_tensor(
                        out=o_sb[:, ct, :],
                        in0=pj,
                        scalar=bf_sb[:, ct : ct + 1],
                        in1=xres,
                        op0=OP.add,
                        op1=OP.add,
                    )
                else:
                    nc.vector.tensor_add(out=o_sb[:, ct, :], in0=pj, in1=xres)
            nc.sync.dma_start(
                out=out[b, :, qc * 512 : (qc + 1) * 512].rearrange(
                    "(t p) n -> p t n", p=P
                ),
                in_=o_sb,
            )

        # --- emission schedule ---
        load_x(0, split=True)
        load_weights()
        for _ in range(N_WARM):
            warm = ps_s.tile([P, 2, 512], F32, tag="s", name="warm_ps")
            nc.tensor.matmul(
                warm[:, 0, :], lhsT=wdum_sb[:, 0:P], rhs=wdum_sb,
                start=True, stop=True,
            )
        phase_stats(0)
        for piece in conv_pieces(0):
            piece()
        pending = None
        for b in range(BL):
            for qc in range(QCH):
                if qc == 0:
                    side = [lambda nb_=b + 1: phase_stats(nb_)] if b + 1 < BL else []
                else:
                    side = conv_pieces(b + 1) if b + 1 < BL else []
                phase_c(b, qc, pending, side)
                pending = (b, qc)
        phase_d(*pending)
    nc.compile()
    return nc


def prepare(inputs):
    """Fold parameters on the host; return (program, per-core input maps)."""
    x = np.asarray(inputs["x"], dtype=np.float32)
    norm_w = np.asarray(inputs["norm_w"], dtype=np.float32)
    norm_b = np.asarray(inputs["norm_b"], dtype=np.float32)
    qkv_w = np.asarray(inputs["qkv_w"], dtype=np.float32)
    qkv_b = np.asarray(inputs["qkv_b"], dtype=np.float32)
    proj_w = np.asarray(inputs["proj_w"], dtype=np.float32)
    proj_b = np.asarray(inputs["proj_b"], dtype=np.float32)

    # Fold the GroupNorm affine into qkv: qkv(h*w+b) = (qkv*w)h + qkv@b
    w_eff = qkv_w * norm_w[None, :]
    b_eff = qkv_b + qkv_w @ norm_b
    f8 = ml_dtypes.float8_e4m3
    bf16 = ml_dtypes.bfloat16
    bq_f = np.ascontiguousarray(b_eff[0:C] * WSCALE)
    bk_f = np.ascontiguousarray(b_eff[C : 2 * C] * WSCALE)
    bv_f = b_eff[2 * C : 3 * C]
    bf_f = np.ascontiguousarray(proj_w @ bv_f + proj_b)
    use_bq = bool(np.any(bq_f))
    use_bk = bool(np.any(bk_f))
    use_bf = bool(np.any(bf_f))

    if use_bq or use_bk:
        wq_t = np.ascontiguousarray((w_eff[0:C] * WSCALE).T.astype(f8))
        wk_t = np.ascontiguousarray((w_eff[C : 2 * C] * WSCALE).T.astype(f8))
    else:
        # M-trick: S^T[kp,q] = h_kp^T (MSCALE Wk^T Wq) h_q; wq carries M
        # in [c_in, c_out] layout directly.
        m_s = MSCALE * (w_eff[C : 2 * C].T @ w_eff[0:C])
        wq_t = np.ascontiguousarray(m_s.astype(f8))
        wk_t = np.ascontiguousarray(np.zeros((C, C), f8))
    wv_t = np.ascontiguousarray((w_eff[2 * C : 3 * C] * WSCALE).T.astype(f8))
    wp_t = np.ascontiguousarray((proj_w * WSCALE).T.astype(f8))

    nc = build_program(use_bq, use_bk, use_bf)

    xr = x.reshape(NCORES, BL, C, N).astype(bf16)
    in_maps = []
    for c in range(NCORES):
        in_maps.append(
            {
                "xs": np.ascontiguousarray(xr[c]),
                "wq": wq_t,
                "wk": wk_t,
                "wv": wv_t,
                "wp": wp_t,
                "bq": bq_f,
                "bk": bk_f,
                "bf": bf_f,
            }
        )
    return nc, in_maps


def run(inputs, trace=False):
    from concourse.bass_utils import run_bass_kernel_spmd

    nc, in_maps = prepare(inputs)
    res = run_bass_kernel_spmd(nc, in_maps, list(range(NCORES)), trace=trace)
    outs = np.stack(
        [np.asarray(res.results[i]["out"]) for i in range(NCORES)]
    )
    full = outs.reshape(B, C, H, W).astype(np.float32)
    return full, res


def kernel(**inputs) -> np.ndarray:
    full, _ = run(inputs, trace=False)
    return full
